# revision 12
# baseline (speedup 1.0000x reference)
"""Direct Conv2d (NCHW, OIHW, VALID, stride 1) on 8 Trainium2 NeuronCores.

Problem: input [16, 4, 512, 512] f32, filter [8, 4, 3, 3] f32
         -> output [16, 8, 510, 510] f32.

Sharding: data-parallel over batch N: 2 images per core, filter replicated.

Per-core algorithm (all shapes hardcoded):
  Output rows are processed in sub-blocks of JB=14 rows. For a sub-block
  starting at output row h0, the conv is expressed as 3 accumulating
  matmuls (one per filter column shift s):

    out[(m,j), w] += sum_{c,q} lhsT_s[(c,q), (m,j)] * in[c, h0+q, w+s]

  where lhsT_s[(c,q), (m,j)] = filter[m, c, q-j, s] for 0 <= q-j < 3.
  K = 4 channels x 16 input rows = 64, M = 8 out-channels x 14 rows = 112,
  N = 510 output columns. The s shift is a free-dim offset into the same
  SBUF tile, so the input is staged exactly once per sub-block.

  Two sub-blocks are packed into the two partition halves of one
  [128, 512] SBUF tile (supertile = 28 output rows); the two input DMAs
  then cover complementary halves of the 16 SDMA ports. The banded
  weight matrices are built host-side from the 288-element filter and
  duplicated into both partition halves.

  PSUM results are copied to SBUF by the vector engine (DMA has no PSUM
  route) and DMA'd to HBM.
"""

import os

os.environ.setdefault("MYCRO_LOCAL_CACHE", "1")

import numpy as np

import concourse.bacc as bacc
import concourse.mybir as mybir
import concourse.tile as tile
from concourse.bass_utils import run_bass_kernel_spmd

N_CORES = 8
IMG_PER_CORE = 2
C_IN, H, W = 4, 512, 512
C_OUT, R, S = 8, 3, 3
HO, WO = 510, 510

JB = 14              # output rows per sub-block
QB = JB + R - 1      # 16 input rows per sub-block
KDIM = C_IN * QB     # 64  (matmul contraction dim)
MDIM = C_OUT * JB    # 112 (matmul output partition dim)
SUPER = 2 * JB       # 28 output rows per supertile (2 sub-blocks)
NSUPER = (HO + SUPER - 1) // SUPER  # 19 (last covers rows 504..509)

# Moving/stationary matmul dtype. float32r (same bits as f32, reduced-
# precision multiply) streams 4x faster on the PE than float32.
DT = mybir.dt.float32r

# Set by test harness: TRACE=True -> capture NTFF profile, LAST_EXEC_NS set.
TRACE = False
TRACE_DIR = None
LAST_EXEC_NS = None
LAST_RESULTS = None

_NC_CACHE = {}


def build_wT(filt: np.ndarray) -> np.ndarray:
    """Banded weight matrices [S, KDIM, MDIM] from filter [8, 4, 3, 3]."""
    wT = np.zeros((S, KDIM, MDIM), np.float32)
    for s in range(S):
        for c in range(C_IN):
            for q in range(QB):
                for m in range(C_OUT):
                    for j in range(JB):
                        r = q - j
                        if 0 <= r < R:
                            wT[s, c * QB + q, m * JB + j] = filt[m, c, r, s]
    return wT


def conv_body(tc, y, x, wt_d):
    nc = tc.nc
    with (
        tc.tile_pool(name="wt", bufs=1) as wt_pool,
        tc.tile_pool(name="xt", bufs=4) as x_pool,
        tc.tile_pool(name="yt", bufs=4) as y_pool,
        tc.tile_pool(name="ps", bufs=6, space="PSUM") as ps_pool,
    ):
        # Weights: [128, 3*112]; partition halves hold identical copies so
        # sub-block 1 (rhs at base partition 64) has aligned lhsT rows.
        wt = wt_pool.tile([128, S * MDIM], DT)
        for s in range(S):
            for half in range(2):
                nc.sync.dma_start(
                    out=wt[64 * half : 64 * half + KDIM, s * MDIM : (s + 1) * MDIM],
                    in_=wt_d[s],
                )
        for i in range(IMG_PER_CORE):
            for B in range(NSUPER):
                # Last supertile overlaps the previous one (rows 482..509;
                # rows 482..503 are recomputed with identical values) so
                # every supertile is full-size with no edge handling.
                h_base = B * SUPER if B < NSUPER - 1 else HO - SUPER
                xt = x_pool.tile([128, W], DT)
                for b in range(2):
                    # dst partition (c*QB+q) <-> src element (c, q, w):
                    # row-major orders match, so flat dst + 3D src is the
                    # same transfer (and CoreSim tracks it correctly).
                    # SWDGE (gpsimd) spreads descriptors across all 16 SDMA
                    # engines by absolute partition; HWDGE caps at 8.
                    nc.gpsimd.dma_start(
                        out=xt[64 * b : 64 * b + KDIM, :],
                        in_=x[i, :, h_base + JB * b : h_base + JB * b + QB, :],
                    )
                for b in range(2):
                    ps = ps_pool.tile([MDIM, WO], mybir.dt.float32)
                    for s in range(S):
                        nc.tensor.matmul(
                            ps[:, :],
                            lhsT=wt[64 * b : 64 * b + KDIM, s * MDIM : (s + 1) * MDIM],
                            rhs=xt[64 * b : 64 * b + KDIM, s : s + WO],
                            start=(s == 0),
                            stop=(s == S - 1),
                        )
                    yt = y_pool.tile([MDIM, WO], mybir.dt.float32)
                    nc.vector.tensor_copy(yt[:, :], ps[:, :])
                    nc.gpsimd.dma_start(
                        out=y[i, :, h_base + JB * b : h_base + JB * b + JB, :],
                        in_=yt[:, :],
                    )


def build_nc(enable_asserts: bool = False):
    nc = bacc.Bacc(
        "TRN2",
        target_bir_lowering=False,
        debug=False,
        enable_asserts=enable_asserts,
        num_devices=N_CORES,
        num_swdge_queues=2,
    )
    x = nc.dram_tensor("x", [IMG_PER_CORE, C_IN, H, W], DT, kind="ExternalInput").ap()
    wt_d = nc.dram_tensor("wt", [S, KDIM, MDIM], DT, kind="ExternalInput").ap()
    y = nc.dram_tensor(
        "y", [IMG_PER_CORE, C_OUT, HO, WO], mybir.dt.float32, kind="ExternalOutput"
    ).ap()
    with tile.TileContext(nc) as tc:
        conv_body(tc, y, x, wt_d)
    nc.compile()
    return nc


def kernel(_input: np.ndarray, _filter: np.ndarray) -> np.ndarray:
    global LAST_EXEC_NS, LAST_RESULTS
    _input = np.ascontiguousarray(np.asarray(_input, dtype=np.float32))
    _filter = np.asarray(_filter, dtype=np.float32)

    key = DT
    if key not in _NC_CACHE:
        _NC_CACHE[key] = build_nc()
    nc = _NC_CACHE[key]

    wT = build_wT(_filter)
    in_maps = [
        {
            "x": np.ascontiguousarray(_input[IMG_PER_CORE * i : IMG_PER_CORE * (i + 1)]),
            "wt": wT,
        }
        for i in range(N_CORES)
    ]
    res = run_bass_kernel_spmd(
        nc, in_maps, list(range(N_CORES)), trace=TRACE, tmpdir=TRACE_DIR
    )
    LAST_EXEC_NS = res.exec_time_ns
    LAST_RESULTS = res
    out = np.concatenate([r["y"] for r in res.results], axis=0)
    return out


# revision 16
# speedup vs baseline: 1.5301x; 1.5301x over previous
"""Direct Conv2d (NCHW, OIHW, VALID, stride 1) on 8 Trainium2 NeuronCores.

Problem: input [16, 4, 512, 512] f32, filter [8, 4, 3, 3] f32
         -> output [16, 8, 510, 510] f32.

Sharding: data-parallel over batch N: 2 images per core, filter replicated.

Per-core algorithm (all shapes hardcoded):
  Output rows are processed in sub-blocks of JB=14 rows. For a sub-block
  starting at output row h0, the conv is expressed as 3 accumulating
  matmuls (one per filter column shift s):

    out[(m,j), w] += sum_{c,q} lhsT_s[(c,q), (m,j)] * in[c, h0+q, w+s]

  where lhsT_s[(c,q), (m,j)] = filter[m, c, q-j, s] for 0 <= q-j < 3.
  K = 4 channels x 16 input rows = 64, M = 8 out-channels x 14 rows = 112,
  N = 510 output columns. The s shift is a free-dim offset into the same
  SBUF tile, so the input is staged exactly once per sub-block.

  Two sub-blocks are packed into the two partition halves of one
  [128, 512] SBUF tile (supertile = 28 output rows); the two input DMAs
  then cover complementary halves of the 16 SDMA ports. The banded
  weight matrices are built host-side from the 288-element filter and
  duplicated into both partition halves.

  PSUM results are copied to SBUF by the vector engine (DMA has no PSUM
  route) and DMA'd to HBM.
"""

import os

os.environ.setdefault("MYCRO_LOCAL_CACHE", "1")

import numpy as np

import concourse.bacc as bacc
import concourse.mybir as mybir
import concourse.tile as tile
from concourse.bass_utils import run_bass_kernel_spmd

N_CORES = 8
IMG_PER_CORE = 2
C_IN, H, W = 4, 512, 512
C_OUT, R, S = 8, 3, 3
HO, WO = 510, 510

JB = 14              # output rows per sub-block
QB = JB + R - 1      # 16 input rows per sub-block
KDIM = C_IN * QB     # 64  (matmul contraction dim)
MDIM = C_OUT * JB    # 112 (matmul output partition dim)
SUPER = 2 * JB       # 28 output rows per supertile (2 sub-blocks)
NSUPER = (HO + SUPER - 1) // SUPER  # 19 (last covers rows 504..509)

# Moving/stationary matmul dtype. float32r (same bits as f32, reduced-
# precision multiply) streams 4x faster on the PE than float32.
DT = mybir.dt.float32r

# Set by test harness: TRACE=True -> capture NTFF profile, LAST_EXEC_NS set.
TRACE = False
TRACE_DIR = None
LAST_EXEC_NS = None
LAST_RESULTS = None

_NC_CACHE = {}


def build_wT(filt: np.ndarray) -> np.ndarray:
    """Banded weight matrices [S, KDIM, MDIM] from filter [8, 4, 3, 3].

    K order is q-major (row = q*C_IN + c) and M order is j-major
    (col = j*C_OUT + m) so the DRAM-side DMA access patterns lead with the
    16-wide q / 14-wide j dims — the HWDGE assigns SDMA engines by the
    outer-dim index of the DRAM AP, so wide outer dims engage all engines.
    """
    wT = np.zeros((S, KDIM, MDIM), np.float32)
    for s in range(S):
        for c in range(C_IN):
            for q in range(QB):
                for m in range(C_OUT):
                    for j in range(JB):
                        r = q - j
                        if 0 <= r < R:
                            wT[s, q * C_IN + c, j * C_OUT + m] = filt[m, c, r, s]
    return wT


def conv_body(tc, y, x, wt_d):
    nc = tc.nc
    with (
        tc.tile_pool(name="wt", bufs=1) as wt_pool,
        tc.tile_pool(name="xt", bufs=4) as x_pool,
        tc.tile_pool(name="yt", bufs=4) as y_pool,
        tc.tile_pool(name="ps", bufs=6, space="PSUM") as ps_pool,
    ):
        # Weights: [128, 3*112]; partition halves hold identical copies so
        # sub-block 1 (rhs at base partition 64) has aligned lhsT rows.
        wt = wt_pool.tile([128, S * MDIM], DT)
        for s in range(S):
            for half in range(2):
                nc.sync.dma_start(
                    out=wt[64 * half : 64 * half + KDIM, s * MDIM : (s + 1) * MDIM],
                    in_=wt_d[s],
                )
        for i in range(IMG_PER_CORE):
            for B in range(NSUPER):
                # Last supertile overlaps the previous one (rows 482..509;
                # rows 482..503 are recomputed with identical values) so
                # every supertile is full-size with no edge handling.
                h_base = B * SUPER if B < NSUPER - 1 else HO - SUPER
                xt = x_pool.tile([128, W], DT)
                for b in range(2):
                    # dst partition (q*C_IN+c) <-> src element (q, c, w):
                    # transpose puts the 16-wide q dim outermost in the DRAM
                    # AP so the transfer spreads over 16 SDMA engines.
                    eng = nc.sync if b == 0 else nc.scalar
                    eng.dma_start(
                        out=xt[64 * b : 64 * b + KDIM, :],
                        in_=x[i, :, h_base + JB * b : h_base + JB * b + QB, :].transpose(
                            [1, 0, 2]
                        ),
                    )
                for b in range(2):
                    ps = ps_pool.tile([MDIM, WO], mybir.dt.float32)
                    for s in range(S):
                        nc.tensor.matmul(
                            ps[:, :],
                            lhsT=wt[64 * b : 64 * b + KDIM, s * MDIM : (s + 1) * MDIM],
                            rhs=xt[64 * b : 64 * b + KDIM, s : s + WO],
                            start=(s == 0),
                            stop=(s == S - 1),
                        )
                    yt = y_pool.tile([MDIM, WO], mybir.dt.float32)
                    nc.vector.tensor_copy(yt[:, :], ps[:, :])
                    # src partition (j*C_OUT+m) <-> dst element (j, m, w):
                    # 14-wide j dim outermost -> 14 SDMA engines.
                    eng = nc.scalar if b == 0 else nc.sync
                    eng.dma_start(
                        out=y[i, :, h_base + JB * b : h_base + JB * b + JB, :].transpose(
                            [1, 0, 2]
                        ),
                        in_=yt[:, :],
                    )


def build_nc(enable_asserts: bool = False):
    nc = bacc.Bacc(
        "TRN2",
        target_bir_lowering=False,
        debug=False,
        enable_asserts=enable_asserts,
        num_devices=N_CORES,
    )
    x = nc.dram_tensor("x", [IMG_PER_CORE, C_IN, H, W], DT, kind="ExternalInput").ap()
    wt_d = nc.dram_tensor("wt", [S, KDIM, MDIM], DT, kind="ExternalInput").ap()
    y = nc.dram_tensor(
        "y", [IMG_PER_CORE, C_OUT, HO, WO], mybir.dt.float32, kind="ExternalOutput"
    ).ap()
    with tile.TileContext(nc) as tc:
        conv_body(tc, y, x, wt_d)
    nc.compile()
    return nc


def kernel(_input: np.ndarray, _filter: np.ndarray) -> np.ndarray:
    global LAST_EXEC_NS, LAST_RESULTS
    _input = np.ascontiguousarray(np.asarray(_input, dtype=np.float32))
    _filter = np.asarray(_filter, dtype=np.float32)

    key = DT
    if key not in _NC_CACHE:
        _NC_CACHE[key] = build_nc()
    nc = _NC_CACHE[key]

    wT = build_wT(_filter)
    in_maps = [
        {
            "x": np.ascontiguousarray(_input[IMG_PER_CORE * i : IMG_PER_CORE * (i + 1)]),
            "wt": wT,
        }
        for i in range(N_CORES)
    ]
    res = run_bass_kernel_spmd(
        nc, in_maps, list(range(N_CORES)), trace=TRACE, tmpdir=TRACE_DIR
    )
    LAST_EXEC_NS = res.exec_time_ns
    LAST_RESULTS = res
    out = np.concatenate([r["y"] for r in res.results], axis=0)
    return out


# revision 18
# speedup vs baseline: 1.9314x; 1.2623x over previous
"""Direct Conv2d (NCHW, OIHW, VALID, stride 1) on 8 Trainium2 NeuronCores.

Problem: input [16, 4, 512, 512] f32, filter [8, 4, 3, 3] f32
         -> output [16, 8, 510, 510] f32.

Sharding: data-parallel over batch N: 2 images per core, filter replicated.

Per-core algorithm (all shapes hardcoded):
  Output rows are processed in sub-blocks of JB=14 rows. For a sub-block
  starting at output row h0, the conv is expressed as 3 accumulating
  matmuls (one per filter column shift s):

    out[(m,j), w] += sum_{c,q} lhsT_s[(c,q), (m,j)] * in[c, h0+q, w+s]

  where lhsT_s[(c,q), (m,j)] = filter[m, c, q-j, s] for 0 <= q-j < 3.
  K = 4 channels x 16 input rows = 64, M = 8 out-channels x 14 rows = 112,
  N = 510 output columns. The s shift is a free-dim offset into the same
  SBUF tile, so the input is staged exactly once per sub-block.

  Two sub-blocks are packed into the two partition halves of one
  [128, 512] SBUF tile (supertile = 28 output rows); the two input DMAs
  then cover complementary halves of the 16 SDMA ports. The banded
  weight matrices are built host-side from the 288-element filter and
  duplicated into both partition halves.

  PSUM results are copied to SBUF by the vector engine (DMA has no PSUM
  route) and DMA'd to HBM.
"""

import os

os.environ.setdefault("MYCRO_LOCAL_CACHE", "1")

import numpy as np

import concourse.bacc as bacc
import concourse.mybir as mybir
import concourse.tile as tile
from concourse.bass_utils import run_bass_kernel_spmd

N_CORES = 8
IMG_PER_CORE = 2
C_IN, H, W = 4, 512, 512
C_OUT, R, S = 8, 3, 3
HO, WO = 510, 510

JB = 14              # output rows per sub-block
QB = JB + R - 1      # 16 input rows per sub-block
KDIM = C_IN * QB     # 64  (matmul contraction dim)
MDIM = C_OUT * JB    # 112 (matmul output partition dim)
SUPER = 2 * JB       # 28 output rows per supertile (2 sub-blocks)
NSUPER = (HO + SUPER - 1) // SUPER  # 19 (last covers rows 504..509)

# Moving/stationary matmul dtype. float32r (same bits as f32, reduced-
# precision multiply) streams 4x faster on the PE than float32.
DT = mybir.dt.float32r

# Set by test harness: TRACE=True -> capture NTFF profile, LAST_EXEC_NS set.
TRACE = False
TRACE_DIR = None
LAST_EXEC_NS = None
LAST_RESULTS = None

_NC_CACHE = {}


def build_wT(filt: np.ndarray) -> np.ndarray:
    """Banded weight matrices [S, KDIM, MDIM] from filter [8, 4, 3, 3].

    K order is q-major (row = q*C_IN + c) and M order is j-major
    (col = j*C_OUT + m) so the DRAM-side DMA access patterns lead with the
    16-wide q / 14-wide j dims — the HWDGE assigns SDMA engines by the
    outer-dim index of the DRAM AP, so wide outer dims engage all engines.
    """
    wT = np.zeros((S, KDIM, MDIM), np.float32)
    for s in range(S):
        for c in range(C_IN):
            for q in range(QB):
                for m in range(C_OUT):
                    for j in range(JB):
                        r = q - j
                        if 0 <= r < R:
                            wT[s, q * C_IN + c, j * C_OUT + m] = filt[m, c, r, s]
    return wT


def conv_body(tc, y, x, wt_d):
    nc = tc.nc
    with (
        tc.tile_pool(name="wt", bufs=1) as wt_pool,
        tc.tile_pool(name="xt", bufs=6) as x_pool,
        tc.tile_pool(name="yt", bufs=6) as y_pool,
        tc.tile_pool(name="ps", bufs=8, space="PSUM") as ps_pool,
    ):
        # Weights: [128, 3*112]; partition halves hold identical copies so
        # sub-block 1 (rhs at base partition 64) has aligned lhsT rows.
        wt = wt_pool.tile([128, S * MDIM], DT)
        for s in range(S):
            for half in range(2):
                nc.sync.dma_start(
                    out=wt[64 * half : 64 * half + KDIM, s * MDIM : (s + 1) * MDIM],
                    in_=wt_d[s],
                )
        for i in range(IMG_PER_CORE):
            for B in range(NSUPER):
                # Last supertile overlaps the previous one (rows 482..509;
                # rows 482..503 are recomputed with identical values) so
                # every supertile is full-size with no edge handling.
                h_base = B * SUPER if B < NSUPER - 1 else HO - SUPER
                # Last supertile: sub-block 0 (rows 482..495) is fully
                # covered by supertile 17; only sub-block 1 is needed.
                blocks = [0, 1] if B < NSUPER - 1 else [1]
                xt = x_pool.tile([128, W], DT)
                for b in blocks:
                    # dst partition (q*C_IN+c) <-> src element (q, c, w):
                    # transpose puts the 16-wide q dim outermost in the DRAM
                    # AP so the transfer spreads over 16 SDMA engines.
                    # gpsimd = SWDGE: third descriptor generator alongside
                    # the two HWDGE rings used by the output DMAs.
                    nc.gpsimd.dma_start(
                        out=xt[64 * b : 64 * b + KDIM, :],
                        in_=x[i, :, h_base + JB * b : h_base + JB * b + QB, :].transpose(
                            [1, 0, 2]
                        ),
                    )
                for b in blocks:
                    ps = ps_pool.tile([MDIM, WO], mybir.dt.float32)
                    for s in range(S):
                        nc.tensor.matmul(
                            ps[:, :],
                            lhsT=wt[64 * b : 64 * b + KDIM, s * MDIM : (s + 1) * MDIM],
                            rhs=xt[64 * b : 64 * b + KDIM, s : s + WO],
                            start=(s == 0),
                            stop=(s == S - 1),
                        )
                    yt = y_pool.tile([MDIM, WO], mybir.dt.float32)
                    nc.vector.tensor_copy(yt[:, :], ps[:, :])
                    # src partition (j*C_OUT+m) <-> dst element (j, m, w):
                    # 14-wide j dim outermost -> 14 SDMA engines.
                    eng = nc.scalar if b == 0 else nc.sync
                    eng.dma_start(
                        out=y[i, :, h_base + JB * b : h_base + JB * b + JB, :].transpose(
                            [1, 0, 2]
                        ),
                        in_=yt[:, :],
                    )


def build_nc(enable_asserts: bool = False):
    nc = bacc.Bacc(
        "TRN2",
        target_bir_lowering=False,
        debug=False,
        enable_asserts=enable_asserts,
        num_devices=N_CORES,
    )
    x = nc.dram_tensor("x", [IMG_PER_CORE, C_IN, H, W], DT, kind="ExternalInput").ap()
    wt_d = nc.dram_tensor("wt", [S, KDIM, MDIM], DT, kind="ExternalInput").ap()
    y = nc.dram_tensor(
        "y", [IMG_PER_CORE, C_OUT, HO, WO], mybir.dt.float32, kind="ExternalOutput"
    ).ap()
    with tile.TileContext(nc) as tc:
        conv_body(tc, y, x, wt_d)
    nc.compile()
    return nc


def kernel(_input: np.ndarray, _filter: np.ndarray) -> np.ndarray:
    global LAST_EXEC_NS, LAST_RESULTS
    _input = np.ascontiguousarray(np.asarray(_input, dtype=np.float32))
    _filter = np.asarray(_filter, dtype=np.float32)

    key = DT
    if key not in _NC_CACHE:
        _NC_CACHE[key] = build_nc()
    nc = _NC_CACHE[key]

    wT = build_wT(_filter)
    in_maps = [
        {
            "x": np.ascontiguousarray(_input[IMG_PER_CORE * i : IMG_PER_CORE * (i + 1)]),
            "wt": wT,
        }
        for i in range(N_CORES)
    ]
    res = run_bass_kernel_spmd(
        nc, in_maps, list(range(N_CORES)), trace=TRACE, tmpdir=TRACE_DIR
    )
    LAST_EXEC_NS = res.exec_time_ns
    LAST_RESULTS = res
    out = np.concatenate([r["y"] for r in res.results], axis=0)
    return out


# revision 22
# speedup vs baseline: 2.4154x; 1.2506x over previous
"""Direct Conv2d (NCHW, OIHW, VALID, stride 1) on 8 Trainium2 NeuronCores.

Problem: input [16, 4, 512, 512] f32, filter [8, 4, 3, 3] f32
         -> output [16, 8, 510, 510] f32.

Sharding: data-parallel over batch N: 2 images per core, filter replicated.

Per-core algorithm (all shapes hardcoded):
  Output rows are processed in sub-blocks of JB=14 rows. For a sub-block
  starting at output row h0, the conv is expressed as 3 accumulating
  matmuls (one per filter column shift s):

    out[(m,j), w] += sum_{c,q} lhsT_s[(c,q), (m,j)] * in[c, h0+q, w+s]

  where lhsT_s[(c,q), (m,j)] = filter[m, c, q-j, s] for 0 <= q-j < 3.
  K = 4 channels x 16 input rows = 64, M = 8 out-channels x 14 rows = 112,
  N = 510 output columns. The s shift is a free-dim offset into the same
  SBUF tile, so the input is staged exactly once per sub-block.

  Two sub-blocks are packed into the two partition halves of one
  [128, 512] SBUF tile (supertile = 28 output rows); the two input DMAs
  then cover complementary halves of the 16 SDMA ports. The banded
  weight matrices are built host-side from the 288-element filter and
  duplicated into both partition halves.

  PSUM results are copied to SBUF by the vector engine (DMA has no PSUM
  route) and DMA'd to HBM.
"""

import os

os.environ.setdefault("MYCRO_LOCAL_CACHE", "1")

import numpy as np

import concourse.bacc as bacc
import concourse.mybir as mybir
import concourse.tile as tile
from concourse.bass_utils import run_bass_kernel_spmd

N_CORES = 8
IMG_PER_CORE = 2
C_IN, H, W = 4, 512, 512
C_OUT, R, S = 8, 3, 3
HO, WO = 510, 510

JB = 14              # output rows per sub-block
QB = JB + R - 1      # 16 input rows per sub-block
KDIM = C_IN * QB     # 64  (matmul contraction dim)
MDIM = C_OUT * JB    # 112 (matmul output partition dim)
SUPER = 2 * JB       # 28 output rows per supertile (2 sub-blocks)
NSUPER = (HO + SUPER - 1) // SUPER  # 19 (last covers rows 504..509)

# Moving/stationary matmul dtype. float32r (same bits as f32, reduced-
# precision multiply) streams 4x faster on the PE than float32.
DT = mybir.dt.float32r

# Set by test harness: TRACE=True -> capture NTFF profile, LAST_EXEC_NS set.
TRACE = False
TRACE_DIR = None
LAST_EXEC_NS = None
LAST_RESULTS = None

_NC_CACHE = {}


def build_wT(filt: np.ndarray) -> np.ndarray:
    """Banded weight matrices [S, 2, 128, MDIM] from filter [8, 4, 3, 3].

    Sub-block beta computes output rows h0 + 2j + beta (row-interleaved),
    so one SBUF partition (j, m) ends up holding two consecutive output
    rows -> 4 KB contiguous HBM chunks on the store.

    K order is q-major (row = q*C_IN + c, q in [0,32)) and M order is
    j-major (col = j*C_OUT + m) so the DRAM-side DMA access patterns lead
    with wide outer dims — the HWDGE assigns SDMA engines by the outer-dim
    index of the DRAM AP.
    """
    wT = np.zeros((S, 2, 128, MDIM), np.float32)
    for s in range(S):
        for beta in range(2):
            for c in range(C_IN):
                for q in range(2 * JB + R + 1):
                    for m in range(C_OUT):
                        for j in range(JB):
                            r = q - 2 * j - beta
                            if 0 <= r < R:
                                wT[s, beta, q * C_IN + c, j * C_OUT + m] = filt[
                                    m, c, r, s
                                ]
    return wT


def conv_body(tc, y, x, wt_d):
    nc = tc.nc
    with (
        tc.tile_pool(name="wt", bufs=1) as wt_pool,
        tc.tile_pool(name="xt", bufs=6) as x_pool,
        tc.tile_pool(name="yt", bufs=6) as y_pool,
        tc.tile_pool(name="ps", bufs=8, space="PSUM") as ps_pool,
    ):
        # Weights: [128, 6*112], one chunk per (s, beta).
        wt = wt_pool.tile([128, S * 2 * MDIM], DT)
        for s in range(S):
            for beta in range(2):
                k = s * 2 + beta
                nc.sync.dma_start(
                    out=wt[:, k * MDIM : (k + 1) * MDIM],
                    in_=wt_d[s, beta],
                )
        for i in range(IMG_PER_CORE):
            for B in range(NSUPER):
                # Last supertile overlaps the previous one (rows 482..509;
                # rows 482..503 are recomputed with identical values) so
                # every supertile is full-size with no edge handling.
                h_base = B * SUPER if B < NSUPER - 1 else HO - SUPER
                nq = min(32, H - h_base)  # last supertile: 30 input rows
                xt = x_pool.tile([128, W], DT)
                # dst partition (q*C_IN+c) <-> src element (q, c, w): the
                # 32-wide q dim outermost spreads over all 16 SDMA engines.
                # gpsimd = SWDGE: third descriptor generator alongside the
                # two HWDGE rings used by the output DMAs.
                nc.gpsimd.dma_start(
                    out=xt[0 : nq * C_IN, :],
                    in_=x[i, :, h_base : h_base + nq, :].transpose([1, 0, 2]),
                )
                yt = y_pool.tile([MDIM, 2 * WO], mybir.dt.float32)
                for b in range(2):
                    ps = ps_pool.tile([MDIM, WO], mybir.dt.float32)
                    kq = nq * C_IN  # 120 on the last supertile (zero-weight
                    # rows beyond the loaded window are simply dropped)
                    for s in range(S):
                        k = s * 2 + b
                        nc.tensor.matmul(
                            ps[:, :],
                            lhsT=wt[0:kq, k * MDIM : (k + 1) * MDIM],
                            rhs=xt[0:kq, s : s + WO],
                            start=(s == 0),
                            stop=(s == S - 1),
                        )
                    # partition (j,m): even rows land in cols [0,510),
                    # odd rows in [510,1020) -> 4080B contiguous HBM chunk.
                    nc.vector.tensor_copy(yt[:, b * WO : (b + 1) * WO], ps[:, :])
                # dst element (j, m, (beta,w)) <-> src partition (j*8+m),
                # free (beta,w); 14-wide j outermost -> 14 SDMA engines.
                eng = nc.scalar if B % 2 == 0 else nc.sync
                eng.dma_start(
                    out=y[i, :, h_base : h_base + SUPER, :].rearrange(
                        "m (j b) w -> j m (b w)", b=2
                    ),
                    in_=yt[:, :],
                )


def build_nc(enable_asserts: bool = False):
    nc = bacc.Bacc(
        "TRN2",
        target_bir_lowering=False,
        debug=False,
        enable_asserts=enable_asserts,
        num_devices=N_CORES,
    )
    x = nc.dram_tensor("x", [IMG_PER_CORE, C_IN, H, W], DT, kind="ExternalInput").ap()
    wt_d = nc.dram_tensor("wt", [S, 2, 128, MDIM], DT, kind="ExternalInput").ap()
    y = nc.dram_tensor(
        "y", [IMG_PER_CORE, C_OUT, HO, WO], mybir.dt.float32, kind="ExternalOutput"
    ).ap()
    with tile.TileContext(nc) as tc:
        conv_body(tc, y, x, wt_d)
    nc.compile()
    return nc


def kernel(_input: np.ndarray, _filter: np.ndarray) -> np.ndarray:
    global LAST_EXEC_NS, LAST_RESULTS
    _input = np.ascontiguousarray(np.asarray(_input, dtype=np.float32))
    _filter = np.asarray(_filter, dtype=np.float32)

    key = DT
    if key not in _NC_CACHE:
        _NC_CACHE[key] = build_nc()
    nc = _NC_CACHE[key]

    wT = build_wT(_filter)
    in_maps = [
        {
            "x": np.ascontiguousarray(_input[IMG_PER_CORE * i : IMG_PER_CORE * (i + 1)]),
            "wt": wT,
        }
        for i in range(N_CORES)
    ]
    res = run_bass_kernel_spmd(
        nc, in_maps, list(range(N_CORES)), trace=TRACE, tmpdir=TRACE_DIR
    )
    LAST_EXEC_NS = res.exec_time_ns
    LAST_RESULTS = res
    out = np.concatenate([r["y"] for r in res.results], axis=0)
    return out
